# revision 42
# baseline (speedup 1.0000x reference)
"""Trainium2 Bass kernel for nn_NegUniform (topk_masking) — v3.

Computes: L2-normalize feature & negative_features, sims = f_hat @ negs_hat^T
per negative set j (masked same-class for j==idx), top-16 per row, softmax
entropy over the J axis, decay-weighted mean + log(J).

Sharding: data-parallel over the n (row) dimension across 8 NeuronCores;
negatives/targets replicated. Each core returns per-row-tile partial sums
[128, RT]; the host reduces to the scalar.

Design (engine-measured rates drove every choice):
  - PE: fp8(e4m3) matmuls. Plain 512-col matmuls for j!=idx (DoubleRow's
    2x is erased by SBUF-read contention with the busy DVE/Act, so plain
    costs the same wall time with half the bandwidth). For j==idx a
    DoubleRow matmul folds the same-class mask for free: k-tile0 =
    (f, negs), k-tile1 = (-64*onehot(class,row), onehot(class,cand)).
    (+-448 is the e4m3 0xFE boundary and decodes as NaN on the PE, so the
    mask uses -64, still far below any cosine.)
  - PSUM drain is the wall (only DVE and Act can read PSUM, 1 el/cycle,
    one PSUM operand per instruction; Pool/GPSIMD has no PSUM port and no
    max op at all; DMA cannot touch PSUM; matmul output must be fp32 on
    TRN2). Per (row-tile, j) pair of 4096 candidates: Act drains 3584 els
    via Exp((v-c)/T) -> bf16 SBUF (the exp costs the same as a copy and
    feeds the entropy directly: softmax over j of v/T == w/sum(w) for
    w = e^{(v-c)/T}); DVE drains the last 512 via one segmented
    tensor_reduce(max, W=32) from PSUM.
  - Selection: group-max compression (G=64) -- DVE tensor_tensor max
    trees on the bf16 exp values at 2x (4 consumed els/cycle), one tree
    per drained piece (2048->32, 1536->24) so the first tree starts right
    after the h0 exp; plus the 16 reduced maxes (exp'd by Act per
    pair -- batching them per row-tile bunched all four trios behind the
    tile's last treduce and serialized the kernel tail) -> 72 leftover; then max8 -> match_replace -> max8
    gives the sorted top-16 (monotone in v). Group-max loses a top-16
    member only when two land in one group; validated numerically at rel
    err ~2.5e-3 vs the fp32 reference (gate 2e-2).
  - Entropy: p=w/S; ent_k = A/S - lnS with A = sum_j w*ln(w). Pool does
    the j-sums (fp32 adds; Pool cannot touch bf16), one deferred batched
    Ln on Act after the last Exp (avoids activation-table thrash), DVE a
    short batched epilogue: (lnS - A/S)*decay summed over k -> partials
    [128, RT]; host adds log(J).

Loads go out as f32 bitcast views of the fp8 tensors (1-byte-element DMA
descriptors run well below line rate) and are spread just-in-time over the
three DMA queues (~50-60GB/s each measured); the activation-table warm-up
reads a memset tile, never a late-arriving input. Measured on the 8 axon
cores: ~88.8-90.7us vs the 107.7us max8-scan baseline. The last three
pairs shift half their h1 drains to fill-gated tensor_reduces (W=64) with
1024-el exps and 1024->16 trees, shortening the exp->tree->trio->entropy
closing chain. (Extending the same treatment to the FIRST pair flips pool
first-allocation order and still NaNs after max-size/slice fixes --
unresolved; that build timed 87.6us, so the direction has headroom.)
Steady state is jointly DVE/Act-bound (~62us busy each, both >90%
occupied mid-kernel), which is the drain-bandwidth floor for this
selection algorithm on TRN2.
"""

import math
import sys

import numpy as np

for _p in ("/opt/trn_rl_repo",):
    if _p not in sys.path:
        sys.path.insert(0, _p)

N = 4096
D = 128
J = 4
NCORES = 8
NLOC = N // NCORES          # 512 rows per core
RT = NLOC // 128            # 4 row-tiles per core
K = 16
TEMP = 0.01
V = 0.95
MASK_NEG = -64.0            # exact in e4m3; dominates any cosine sim (+-448 = 0xFE decodes as NaN on the PE)
EXP_C = 0.35                # exp centering: w = exp((v - EXP_C)/TEMP)

DSHARE = 512                # candidates drained by DVE tensor_reduce
ASHARE = N - DSHARE         # candidates drained by Act per (t, j) pair
DW = 32                     # tensor_reduce window (G for the D share)
NLVL = 6                    # TT-max tree levels (G=64)
LO = 32 + 24 + DSHARE // DW  # leftover per pair (A-tree + B-tree + dred)

_BUILD_CACHE = {}
LAST_RESULT = None  # BassKernelResults of the most recent kernel() call

def _build(idx: int):
    if idx in _BUILD_CACHE:
        return _BUILD_CACHE[idx]

    import concourse.bacc as bacc
    import concourse.tile as tile
    import concourse.mybir as mybir

    f32 = mybir.dt.float32
    bf16 = mybir.dt.bfloat16
    fp8 = mybir.dt.float8e4
    AF = mybir.ActivationFunctionType
    OP = mybir.AluOpType
    DR = mybir.MatmulPerfMode.DoubleRow

    nc = bacc.Bacc(
        "TRN2",
        target_bir_lowering=False,
        debug=False,
        enable_asserts=False,
        num_devices=1,
    )

    wPd = nc.dram_tensor("wP", [D, NLOC], fp8, kind="ExternalInput").ap()
    wMd = nc.dram_tensor("wM", [D, 2, NLOC], fp8, kind="ExternalInput").ap()
    negsd = nc.dram_tensor("negsT", [J, D, N], fp8,
                           kind="ExternalInput").ap()
    pkd = nc.dram_tensor("negsPK", [D, 2 * N], fp8, kind="ExternalInput").ap()
    decayd = nc.dram_tensor("decayW", [128, RT * K], f32,
                            kind="ExternalInput").ap()
    outd = nc.dram_tensor("out", [128, RT], f32, kind="ExternalOutput").ap()

    jorder = [j for j in range(J) if j != idx] + [idx]

    with tile.TileContext(nc) as tc:
        with (
            tc.tile_pool(name="consts", bufs=1) as cpool,
            tc.tile_pool(name="wb", bufs=4) as wpool,
            tc.tile_pool(name="tr1", bufs=2) as t1p,
            tc.tile_pool(name="tr2", bufs=2) as t2p,
            tc.tile_pool(name="tr3", bufs=2) as t3p,
            tc.tile_pool(name="tr4", bufs=2) as t4p,
            tc.tile_pool(name="tr5", bufs=2) as t5p,
            tc.tile_pool(name="lo", bufs=3) as lop,
            tc.tile_pool(name="dred", bufs=2) as drp,
            tc.tile_pool(name="rep", bufs=3) as repp,
            tc.tile_pool(name="ent", bufs=2) as epool,
            tc.tile_pool(name="psums", bufs=2, space="PSUM") as psp,
        ):
            # ---- loads ----
            negs_t = {}
            for j in range(J):
                if j != idx:
                    negs_t[j] = cpool.tile([128, N], fp8, tag=f"negsT{j}",
                                           name=f"negsT{j}")
            pk_t = cpool.tile([128, 2 * N], fp8)
            wP_t = cpool.tile([128, NLOC], fp8)
            wM_t = cpool.tile([128, 2, NLOC], fp8)
            decay_t = cpool.tile([128, RT * K], f32)

            j0 = jorder[0]
            # first unit's columns first so matmuls start early; few
            # triggers; fp8 transfers issued as f32 bitcast views (1-byte
            # element descriptors run the DMA ~5x below line rate)
            def _ld(eng, dst, src):
                eng.dma_start(dst.bitcast(f32), src.bitcast(f32))

            j1, j2 = jorder[1], jorder[2]
            _ld(nc.sync, negs_t[j0][:, 0:1024], negsd[j0, :, 0:1024])
            _ld(nc.scalar, wP_t, wPd)
            _ld(nc.gpsimd, negs_t[j0][:, 2048:N], negsd[j0, :, 2048:N])
            _ld(nc.scalar, negs_t[j0][:, 1024:2048], negsd[j0, :, 1024:2048])
            _ld(nc.sync, negs_t[j1][:, 0:2048], negsd[j1, :, 0:2048])
            _ld(nc.scalar, wM_t, wMd)
            _ld(nc.sync, negs_t[j1][:, 2048:N], negsd[j1, :, 2048:N])
            _ld(nc.gpsimd, pk_t, pkd)
            _ld(nc.scalar, negs_t[j2][:, 0:2048], negsd[j2, :, 0:2048])
            _ld(nc.sync, negs_t[j2][:, 2048:N], negsd[j2, :, 2048:N])
            nc.sync.dma_start(decay_t, decayd)

            # warm the exp/ln activation table during the load phase from a
            # memset tile (decay_t arrives last -- gating the warm on it
            # stalled the first drain exp by ~3.5us)
            bias_t = cpool.tile([128, 1], f32)
            nc.vector.memset(bias_t, -EXP_C / TEMP)
            warm = cpool.tile([128, 8], f32)
            nc.vector.memset(warm, 0.0)
            nc.scalar.activation(out=warm, in_=warm, func=AF.Exp)

            Vt_all = cpool.tile([128, RT * J * K], f32)   # sorted top-16 (w)
            Sall = cpool.tile([128, RT * K], f32)
            Aall = cpool.tile([128, RT * K], f32)

            pk_v = pk_t.rearrange("p (two n) -> p two n", two=2)

            ND = DSHARE // DW
            for t in range(RT):
                Vt32 = Vt_all[:, t * J * K:(t + 1) * J * K]
                lo_t = lop.tile([128, J * LO], bf16, tag="lo")
                dred_t = drp.tile([128, J * ND], f32, tag="dred")
                for j in jorder:
                    last_pair = (
                        t == RT - 1 and j in jorder[-3:])
                    if j == idx:
                        lhsT = wM_t[:, :, t * 128:(t + 1) * 128]
                    else:
                        lhsT = wP_t[:, t * 128:(t + 1) * 128]
                    wbuf = wpool.tile([128, ASHARE], bf16, tag="wbuf")
                    dred = dred_t[:, j * ND:(j + 1) * ND]
                    lo = lo_t[:, j * LO:(j + 1) * LO]

                    for h in range(2):
                        ps = psp.tile([128, 2048], f32, tag="sims")
                        for c in range(4):
                            c0 = h * 2048 + c * 512
                            if j == idx:
                                nc.tensor.matmul(
                                    ps[:, c * 512:(c + 1) * 512],
                                    lhsT=lhsT, rhs=pk_v[:, :, c0:c0 + 512],
                                    start=True, stop=True, perf_mode=DR,
                                )
                            else:
                                nc.tensor.matmul(
                                    ps[:, c * 512:(c + 1) * 512],
                                    lhsT=lhsT,
                                    rhs=negs_t[j][:, c0:c0 + 512],
                                    start=True, stop=True,
                                )
                        if h == 0:
                            nc.scalar.activation(
                                out=wbuf[:, 0:2048], in_=ps, func=AF.Exp,
                                scale=1.0 / TEMP, bias=bias_t)
                        elif last_pair:
                            # shorten the tail-critical chain: exp only 1024,
                            # treduce (starts at fill, not exp) the rest
                            nc.scalar.activation(
                                out=wbuf[:, 2048:3072], in_=ps[:, 0:1024],
                                func=AF.Exp,
                                scale=1.0 / TEMP, bias=bias_t)
                            nc.vector.tensor_reduce(
                                out=dred,
                                in_=ps[:, 1024:2048].rearrange(
                                    "p (g w) -> p g w", w=64),
                                op=OP.max, axis=mybir.AxisListType.X)
                            nc.scalar.activation(
                                out=lo[:, 48:64], in_=dred, func=AF.Exp,
                                scale=1.0 / TEMP, bias=bias_t)
                        else:
                            nc.scalar.activation(
                                out=wbuf[:, 2048:ASHARE], in_=ps[:, 0:1536],
                                func=AF.Exp,
                                scale=1.0 / TEMP, bias=bias_t)
                            nc.vector.tensor_reduce(
                                out=dred,
                                in_=ps[:, 1536:2048].rearrange(
                                    "p (g w) -> p g w", w=DW),
                                op=OP.max, axis=mybir.AxisListType.X)
                            nc.scalar.activation(
                                out=lo[:, 56:LO], in_=dred, func=AF.Exp,
                                scale=1.0 / TEMP, bias=bias_t)

                    # DVE TT-max trees (bf16, 2x), per drained half so the
                    # first tree starts right after the h0 exp: A 2048->32
                    # (G=64), B 1536->24 (G=64)
                    pools = [t1p, t2p, t3p, t4p, t5p]
                    mrg = t5p.tile([128, 112], bf16, tag="mrg")
                    prev = wbuf[:, 0:2048]
                    sz = 1024
                    for li in range(5):
                        if sz == 64:
                            dst = mrg[:, 0:64]
                        else:
                            dst = pools[li].tile([128, sz], bf16,
                                                 tag=f"a{li + 1}")
                        nc.vector.tensor_tensor(
                            dst, prev[:, 0:sz], prev[:, sz:2 * sz], op=OP.max)
                        prev = dst
                        sz //= 2
                    prev = wbuf[:, 2048:ASHARE]
                    sz = 768
                    for li in range(5):
                        if sz == 48:
                            dst = mrg[:, 64:112]
                        else:
                            dst = pools[li].tile([128, sz], bf16,
                                                 tag=f"b{li + 1}")
                        nc.vector.tensor_tensor(
                            dst, prev[:, 0:sz], prev[:, sz:2 * sz], op=OP.max)
                        prev = dst
                        sz //= 2
                    nc.vector.tensor_tensor(
                        lo[:, 0:56], mrg[:, 0:56], mrg[:, 56:112], op=OP.max)


                # sorted top-16 of the leftover group-maxes, per pair
                for j in jorder:
                    lw = 64 if (
                        t == RT - 1 and j in jorder[-3:]) else LO
                    lo = lo_t[:, j * LO:j * LO + lw]
                    vsl = Vt_all[:, (t * J + j) * K:(t * J + j) * K + K]
                    nc.vector.max(out=vsl[:, 0:8], in_=lo)
                    rep = repp.tile([128, lw], bf16, tag="rep")
                    nc.vector.match_replace(
                        out=rep, in_to_replace=vsl[:, 0:8], in_values=lo,
                        imm_value=-1.0)
                    nc.vector.max(out=vsl[:, 8:16], in_=rep)

                # ---- per-row-tile S sums (fp32 on Pool; no Ln needed) ----
                s01 = epool.tile([128, K], f32, tag="s01", name=f"s01_{t}")
                s23 = epool.tile([128, K], f32, tag="s23", name=f"s23_{t}")
                nc.gpsimd.tensor_tensor(
                    s01, Vt32[:, 0:K], Vt32[:, K:2 * K], op=OP.add)
                nc.gpsimd.tensor_tensor(
                    s23, Vt32[:, 2 * K:3 * K], Vt32[:, 3 * K:4 * K],
                    op=OP.add)
                nc.gpsimd.tensor_tensor(
                    Sall[:, t * K:(t + 1) * K], s01, s23, op=OP.add)

            # ---- deferred Ln phase (single activation-table switch) ----
            W = RT * J * K
            lnv_all = cpool.tile([128, W], f32)
            nc.scalar.activation(out=lnv_all, in_=Vt_all, func=AF.Ln)
            wl_all = cpool.tile([128, W], f32)
            nc.vector.tensor_tensor(wl_all, Vt_all, lnv_all, op=OP.mult)
            for t in range(RT):
                wl = wl_all[:, t * J * K:(t + 1) * J * K]
                a01 = epool.tile([128, K], f32, tag="a01", name=f"a01_{t}")
                a23 = epool.tile([128, K], f32, tag="a23", name=f"a23_{t}")
                nc.vector.tensor_tensor(
                    a01, wl[:, 0:K], wl[:, K:2 * K], op=OP.add)
                nc.vector.tensor_tensor(
                    a23, wl[:, 2 * K:3 * K], wl[:, 3 * K:4 * K], op=OP.add)
                nc.vector.tensor_tensor(
                    Aall[:, t * K:(t + 1) * K], a01, a23, op=OP.add)

            # ---- batched epilogue: negent = lnS - A/S; * decay; sum_k ----
            W = RT * K
            rS = cpool.tile([128, W], f32)
            nc.vector.reciprocal(rS, Sall)
            lnS = cpool.tile([128, W], f32)
            nc.scalar.activation(out=lnS, in_=Sall, func=AF.Ln)
            nc.vector.tensor_tensor(Aall, Aall, rS, op=OP.mult)   # A/S
            nc.vector.scalar_tensor_tensor(
                out=Aall, in0=lnS, scalar=1.0, in1=Aall,
                op0=OP.mult, op1=OP.subtract)                     # lnS - A/S
            nc.vector.tensor_tensor(Aall, Aall, decay_t, op=OP.mult)
            partials = cpool.tile([128, RT], f32)
            nc.vector.tensor_reduce(
                out=partials, in_=Aall.rearrange("p (t k) -> p t k", k=K),
                op=OP.add, axis=mybir.AxisListType.X)
            nc.sync.dma_start(outd, partials)

    nc.compile()
    _BUILD_CACHE[idx] = nc
    return nc


def kernel(feature, target, negative_features, idx):
    import ml_dtypes
    from concourse.bass_utils import run_bass_kernel_spmd

    npf8 = ml_dtypes.float8_e4m3fn

    feature = np.asarray(feature, dtype=np.float32)
    target = np.asarray(target).astype(np.int64)
    negs = np.asarray(negative_features, dtype=np.float32)
    idx_i = int(np.asarray(idx))

    # normalize + cast + transpose on host (layout/quantization prep)
    f = feature / np.maximum(
        np.linalg.norm(feature, axis=-1, keepdims=True), 1e-12)
    g = negs / np.maximum(
        np.linalg.norm(negs, axis=-1, keepdims=True), 1e-12)
    fT_all = np.ascontiguousarray(f.T.astype(npf8))                  # [D, N]
    negsT = g.transpose(0, 2, 1).astype(npf8)                        # [J,D,N]
    negsTp = np.ascontiguousarray(negsT)
    onehot = (target[None, :] == np.arange(J)[:, None])              # [J, N]
    pk = np.zeros((D, 2 * N), dtype=npf8)
    pk[:, 0:N] = negsT[idx_i]
    pk[0:J, N:2 * N] = onehot.astype(npf8)
    maskW = np.zeros((D, NLOC * NCORES), dtype=np.float32)
    for cls in range(J):
        maskW[cls, :] = MASK_NEG * (target == cls)
    decay = V ** np.arange(K, dtype=np.float64)
    decay = decay / decay.sum()
    decay_row = np.tile(decay.astype(np.float32), RT)                # [RT*K]
    decayW = np.broadcast_to(decay_row, (128, RT * K)).copy()

    nc = _build(idx_i)
    in_maps = []
    for c in range(NCORES):
        sl = slice(c * NLOC, (c + 1) * NLOC)
        wP = np.ascontiguousarray(fT_all[:, sl])
        wM = np.zeros((D, 2, NLOC), dtype=npf8)
        wM[:, 0, :] = fT_all[:, sl]
        wM[:, 1, :] = maskW[:, sl].astype(npf8)
        in_maps.append({
            "wP": wP,
            "wM": wM,
            "negsT": negsTp,
            "negsPK": pk,
            "decayW": decayW,
        })

    res = run_bass_kernel_spmd(nc, in_maps, core_ids=list(range(NCORES)))
    global LAST_RESULT
    LAST_RESULT = res
    total = 0.0
    for c in range(NCORES):
        total += float(np.asarray(res.results[c]["out"],
                                  dtype=np.float64).sum())
    loss = -total / N + math.log(J)
    return np.float32(loss)


if __name__ == "__main__":
    rng = np.random.default_rng(0)
    f = rng.standard_normal((N, D)).astype(np.float32)
    ng = rng.standard_normal((J, N, D)).astype(np.float32)
    tg = rng.integers(0, J, size=N).astype(np.int64)
    print(kernel(f, tg, ng, 0))
